# revision 40
# baseline (speedup 1.0000x reference)
"""Trainium2 Bass kernel for Transformer-XL-style relative-position attention.

Problem (per reference):
  T=512 tokens, B=8 batch, D=512 model dim, H=8 heads, DH=64.
  energy = (q+u)@k^T + (q+v)@rpe^T(rel) ; rpe = sinusoidal(i-j) @ W_pos
  softmax over j (diag masked), out = (attn@v) @ W_out + b_out.

Strategy:
  - Data parallel over batch: core b computes batch element b end-to-end.
    No collectives needed.
  - The (t,t,d) rpe tensor is never materialized. Using
    sin((i-j)f) = sin(if)cos(jf) - cos(if)sin(jf) (and the cos analog),
    the BD term factorizes exactly into plain matmuls:
      P^T   = W_pos_h^T @ (q+v)^T            (per head, contraction 64)
      C1    = sin(if).P_sin + cos(if).P_cos  (elementwise, DVE)
      C2    = sin(if).P_cos - cos(if).P_sin
      BD^T  = G^T.T @ [C1;C2]  where G = [cos(jf) | sin(jf)] is constant.
  - Everything runs in feature-major ("transposed") layout (j on
    partitions): energies accumulate in PSUM together with the (q+u)k
    term and a -BIG diagonal-mask matmul; one wide exp pass per j-pair
    produces attn^T.
  - V carries 64 ones-columns per head so the attn@v matmul emits the
    softmax denominator replicated on partitions 64:128; 1/den comes
    from exp(-ln(den)) on the scalar engine; one tensor_tensor divide.
  - bf16 matmul inputs, fp32 PSUM accumulation.
"""

import sys

sys.path.insert(0, "/opt/trn_rl_repo")

import numpy as np
import ml_dtypes

T, B, D, H = 512, 8, 512, 8
DH = D // H
HALF = D // 2
NT = T // 128          # 4 token tiles
ND = D // 128          # 4 feature tiles
NEG_BIG = -30000.0

BF16 = ml_dtypes.bfloat16

_CACHE = {}


def _patch_tile_drain():
    """walrus in this image rejects >1 sync-waits on one TPB_CTRL drain;
    split the TileContext tail-drain waits across several drains."""
    import concourse.tile as tile
    import concourse.mybir as mybir

    if getattr(tile.TileContext, "_drain_patched", False):
        return

    def _drain_and_barrier(self, tick_clock, wait_clock):
        from concourse.vector_clock import ScopedClock

        nc = self.nc
        drain_inst = nc.sync.drain()
        wait_clock.add_sem_waits(
            drain_inst.ins, ScopedClock({None: tick_clock.global_clock})
        )
        si = drain_inst.ins.sync_info
        waits = list(si.on_wait or [])
        if len(waits) > 1:
            si.on_wait[:] = waits[:1]
            for w in waits[1:]:
                extra = nc.sync.drain()
                extra.ins.sync_info = mybir.SyncInfo(on_wait=[w], on_update=[])

        nc.all_engine_barrier()
        assert self.sems is not None
        popped = nc._tile_sem_poison_stack.pop()
        assert popped is self._sem_poison
        nc.clear_and_free_semaphores(list(self.sems.allocated().values()))
        nc.all_engine_barrier()

    tile.TileContext._drain_and_barrier = _drain_and_barrier
    tile.TileContext._drain_patched = True


def _split_multi_waits(nc, limit=1):
    """This walrus build rejects >limit sync-waits on one instruction;
    hoist extra waits onto same-engine NoOp carriers placed just before."""
    import concourse.mybir as mybir

    ctr = [0]
    for f in nc.m.functions:
        for blk in f.blocks:
            new_insts = []
            for inst in blk.instructions:
                si = inst.sync_info
                waits = list(si.on_wait) if si and si.on_wait else []
                if len(waits) > limit:
                    for i in range(limit, len(waits), limit):
                        ctr[0] += 1
                        nop = mybir.InstNoOp(
                            name=f"waitnop{ctr[0]}", ins=[], outs=[]
                        )
                        nop.engine = inst.engine
                        nop.sync_info = mybir.SyncInfo(
                            on_wait=waits[i : i + limit], on_update=[]
                        )
                        new_insts.append(nop)
                    si.on_wait[:] = waits[:limit]
                new_insts.append(inst)
            blk.instructions[:] = new_insts


def _build():
    import concourse.bass as bass
    import concourse.mybir as mybir
    import concourse.tile as tile

    _patch_tile_drain()

    f32 = mybir.dt.float32
    bf16 = mybir.dt.bfloat16
    AF = mybir.ActivationFunctionType

    nc = bass.Bass("TRN2", target_bir_lowering=True, debug=False, num_devices=B)

    with tile.TileContext(nc) as tc:
        # ---- DRAM parameters (coalesced; [p, dt*W + c] = full[dt*128+p, c])
        xT_d = nc.dram_tensor("xT", [128, ND * T], bf16, kind="ExternalInput")
        wqkv_d = nc.dram_tensor(
            "wqkv", [128, ND * 3 * D], bf16, kind="ExternalInput"
        )
        wpT_d = nc.dram_tensor("wpT", [128, ND * D], bf16, kind="ExternalInput")
        wout_d = nc.dram_tensor("wout", [128, ND * D], bf16, kind="ExternalInput")
        gt_d = nc.dram_tensor("gt", [128, ND * T], bf16, kind="ExternalInput")
        # wide trig grids: [p, k*512 + i] = trig(i * freq[k*128 + p])
        sinw_d = nc.dram_tensor("sinw", [128, 2 * T], bf16, kind="ExternalInput")
        cosw_d = nc.dram_tensor("cosw", [128, 2 * T], bf16, kind="ExternalInput")
        # [dneg | id128]
        msk_d = nc.dram_tensor("msk", [128, 256], bf16, kind="ExternalInput")
        # [posu0..3 | posv0..3 | bout0..3 | ltau]
        sml_d = nc.dram_tensor("sml", [128, 13], f32, kind="ExternalInput")
        out_d = nc.dram_tensor("out", [128, ND * T], f32, kind="ExternalOutput")

        # ---- static SBUF tiles -----------------------------------------
        with tc.tile_pool(name="static", bufs=1) as sp:
            xtw = sp.tile([128, ND * T], bf16, name="xtw")
            wqkvw = sp.tile([128, ND * 3 * D], bf16, name="wqkvw")
            wptw = sp.tile([128, ND * D], bf16, name="wptw")
            woutw = sp.tile([128, ND * D], bf16, name="woutw")
            gtw = sp.tile([128, ND * T], bf16, name="gtw")
            sinw = sp.tile([128, 2 * T], bf16, name="sinw")
            cosw = sp.tile([128, 2 * T], bf16, name="cosw")
            msk = sp.tile([128, 256], bf16, name="msk")
            sml = sp.tile([128, 13], f32, name="sml")
            lnm = sp.tile([128, 1], f32, name="lnm")
            mb = sp.tile([128, 1], f32, name="mb")

            quT = [sp.tile([128, T], bf16, name=f"quT{i}") for i in range(ND)]
            qvT = [sp.tile([128, T], bf16, name=f"qvT{i}") for i in range(ND)]
            kT = [sp.tile([128, T], bf16, name=f"kT{i}") for i in range(ND)]
            vsb = [sp.tile([128, 8 * 128], bf16, name=f"v{i}") for i in range(NT)]
            avn = [sp.tile([128, T], bf16, name=f"avn{i}") for i in range(ND)]
            outw = sp.tile([128, ND * T], f32, name="outw")

            nc.sync.dma_start(sml[:], sml_d[:, :])
            nc.sync.dma_start(xtw[:], xT_d[:, :])
            nc.sync.dma_start(wqkvw[:, 0:1024], wqkv_d[:, 0:1024])  # Q01
            nc.sync.dma_start(wptw[:], wpT_d[:, :])
            nc.sync.dma_start(sinw[:], sinw_d[:, :])
            nc.sync.dma_start(cosw[:], cosw_d[:, :])
            nc.sync.dma_start(wqkvw[:, 1024:2048], wqkv_d[:, 1024:2048])  # Q23
            nc.sync.dma_start(wqkvw[:, 2048:4096], wqkv_d[:, 2048:4096])  # K
            nc.sync.dma_start(gtw[:, 0:T], gt_d[:, 0:T])
            nc.sync.dma_start(msk[:], msk_d[:, :])
            nc.sync.dma_start(gtw[:, T:4 * T], gt_d[:, T:4 * T])
            nc.sync.dma_start(wqkvw[:, 4096:6144], wqkv_d[:, 4096:6144])  # V
            nc.sync.dma_start(woutw[:], wout_d[:, :])

            def xT(dt):
                return xtw[:, dt * T:(dt + 1) * T]

            def wqkv_nm(ntile, dt):
                # n-major: [p, ntile*512 + dt*128 + c]
                base = ntile * 512 + dt * 128
                return wqkvw[:, base:base + 128]

            def wpT(dt, c0, c1):
                return wptw[:, dt * D + c0:dt * D + c1]

            def wout(dt, c0, c1):
                return woutw[:, dt * D + c0:dt * D + c1]

            def gt(g, c0, c1):
                return gtw[:, g * T + c0:g * T + c1]

            dneg = msk[:, 0:128]
            dmask = msk[:, 128:256]
            posu = [sml[:, i:i + 1] for i in range(ND)]
            posv = [sml[:, 4 + i:5 + i] for i in range(ND)]
            bout = [sml[:, 8 + i:9 + i] for i in range(ND)]
            ltau = sml[:, 12:13]

            # m = exp(ltau) * DH^-0.5  (log-space fold of the 1/8 scale)
            nc.vector.memset(lnm[:], float(np.log(DH ** -0.5)))
            nc.scalar.activation(mb[:], ltau, AF.Exp, bias=lnm[:], scale=1.0)

            with (
                tc.tile_pool(name="work", bufs=2) as wk,
                tc.tile_pool(name="ps", bufs=2, space="PSUM") as ps,
            ):
                # shared PSUM tags: acc (2x1 bank), p (1x2), e (2x2) = 8
                ps_p = ps_e = ps_av = ps

                def emit_qkv(ntile):
                    acc = ps.tile([128, T], f32, name="qkv_ps", tag="acc", bufs=1)
                    for dt in range(ND):
                        nc.tensor.matmul(
                            acc[:],
                            wqkv_nm(ntile, dt),
                            xT(dt),
                            start=(dt == 0),
                            stop=(dt == ND - 1),
                        )
                    if ntile < 4:
                        nc.scalar.activation(
                            quT[ntile][:], acc[:], AF.Identity,
                            bias=posu[ntile], scale=1.0,
                        )
                        nc.vector.tensor_scalar_add(
                            qvT[ntile][:], acc[:], posv[ntile]
                        )
                    else:
                        nc.scalar.copy(kT[ntile - 4][:], acc[:])

                def emit_v(it):
                    acc = ps.tile([128, D], f32, name="v_ps", tag="acc", bufs=1)
                    for dt in range(ND):
                        vrhs = wqkvw[:].rearrange(
                            "p (nt dt c) -> p nt dt c", nt=12, dt=4
                        )[:, 8:12, dt, :]
                        nc.tensor.matmul(
                            acc[:],
                            xT(dt)[:, it * 128:(it + 1) * 128],
                            vrhs,
                            start=(dt == 0),
                            stop=(dt == ND - 1),
                        )
                    vview = vsb[it][:].rearrange("p (h c) -> p h c", c=128)
                    nc.scalar.copy(
                        vview[:, :, 0:64],
                        acc[:].rearrange("p (h c) -> p h c", c=64),
                    )
                    nc.gpsimd.memset(vview[:, :, 64:128], 1.0)

                # ---- per-head pipeline, software-pipelined ------------
                # P/modulation for head h+2 are interleaved into head h's
                # energy/attention matmuls so the PE never waits on the
                # DVE/GpSimd modulation chain, and modulation reads P
                # straight from PSUM (no copy).
                st = {}

                def emit_p_pair(h, gp):
                    hd_tile = h // 2
                    hd_off = (h % 2) * 64
                    qv_h = qvT[hd_tile][hd_off:hd_off + 64, :]
                    d = st.setdefault(h, {})
                    pacc = ps_p.tile([128, 2 * T], f32, name=f"p_ps{gp}_{h}",
                                     tag="p", bufs=1)
                    for g2 in range(2):
                        g = gp * 2 + g2
                        nc.tensor.matmul(
                            pacc[:, g2 * T:(g2 + 1) * T],
                            wpT(hd_tile, g * 128, (g + 1) * 128)[
                                hd_off:hd_off + 64, :
                            ],
                            qv_h,
                            start=True,
                            stop=True,
                            skip_group_check=True,
                        )
                    ptw = wk.tile([128, 2 * T], bf16, name=f"ptw{gp}_{h}",
                                  tag=f"ptw{gp}", bufs=4)
                    d[f"ptw{gp}"] = ptw
                    if gp == 0:
                        nc.scalar.copy(ptw[:], pacc[:])
                    else:
                        nc.vector.tensor_copy(ptw[:], pacc[:])

                def emit_mods(h):
                    # C1 = sin.Psin + cos.Pcos ; C2 = sin.Pcos - cos.Psin
                    d = st[h]
                    psin, pcos = d["ptw0"], d["ptw1"]
                    ctw = [
                        wk.tile([128, 2 * T], bf16, name=f"ctw{g}_{h}",
                                tag=f"ctw{g}", bufs=4)
                        for g in range(2)
                    ]
                    d["ctw"] = ctw
                    ta = wk.tile([128, 2 * T], bf16, name=f"ta{h}", tag="ta",
                                 bufs=4)
                    tb = wk.tile([128, 2 * T], bf16, name=f"tb{h}", tag="tb",
                                 bufs=4)
                    nc.vector.tensor_mul(ta[:], sinw[:], psin[:])
                    nc.vector.tensor_mul(tb[:], cosw[:], pcos[:])
                    nc.vector.tensor_add(ctw[0][:], ta[:], tb[:])
                    ta2 = wk.tile([128, 2 * T], bf16, name=f"ta2{h}", tag="ta2",
                                  bufs=4)
                    tb2 = wk.tile([128, 2 * T], bf16, name=f"tb2{h}", tag="tb2",
                                  bufs=4)
                    nc.vector.tensor_mul(ta2[:], sinw[:], pcos[:])
                    nc.vector.tensor_mul(tb2[:], cosw[:], psin[:])
                    nc.gpsimd.tensor_sub(ctw[1][:], ta2[:], tb2[:])

                emit_qkv(0)
                emit_qkv(4)
                emit_p_pair(0, 0)
                emit_p_pair(0, 1)
                emit_mods(0)
                emit_p_pair(1, 0)
                emit_p_pair(1, 1)
                emit_mods(1)
                emit_qkv(1)
                emit_qkv(5)
                emit_p_pair(2, 0)
                emit_p_pair(2, 1)
                emit_mods(2)
                for nt_ in (2, 6, 3, 7):
                    emit_qkv(nt_)

                def emit_av_pair(h, attnTw, avacc, jts):
                    for jt in jts:
                        nc.tensor.matmul(
                            avacc[:],
                            vsb[jt][:, h * 128:(h + 1) * 128],
                            attnTw[jt // 2][:, (jt % 2) * T:(jt % 2 + 1) * T],
                            start=(jt == 0),
                            stop=(jt == NT - 1),
                            skip_group_check=True,
                        )

                for h in range(H):
                    hd_tile = h // 2
                    hd_off = (h % 2) * 64
                    qu_h = quT[hd_tile][hd_off:hd_off + 64, :]
                    ctw = st[h]["ctw"]

                    # energy^T per wide j-pair; one wide exp per pair
                    attnTw = [
                        wk.tile([128, 2 * T], bf16, name=f"atw{j}_{h}",
                                tag=f"atw{j}")
                        for j in range(2)
                    ]
                    for jt in range(NT):
                        eacc = ps_e.tile([128, T], f32, name=f"e_ps{jt}",
                                          tag="e", bufs=5)
                        nc.tensor.matmul(
                            eacc[:],
                            kT[hd_tile][hd_off:hd_off + 64,
                                        jt * 128:(jt + 1) * 128],
                            qu_h,
                            start=True,
                            stop=False,
                            skip_group_check=True,
                        )
                        for g in range(ND):
                            nc.tensor.matmul(
                                eacc[:],
                                gt(g, jt * 128, (jt + 1) * 128),
                                ctw[g // 2][:, (g % 2) * T:(g % 2 + 1) * T],
                                start=False,
                                stop=False,
                                skip_group_check=True,
                            )
                        nc.tensor.matmul(
                            eacc[:, jt * 128:(jt + 1) * 128],
                            dneg,
                            dmask,
                            start=False,
                            stop=True,
                            skip_group_check=True,
                        )
                        nc.scalar.activation(
                            attnTw[jt // 2][:, (jt % 2) * T:(jt % 2 + 1) * T],
                            eacc[:], AF.Exp, bias=0.0, scale=mb[:],
                        )

                    # attn@v feature-major; ones rows give the denominator
                    # replicated on partitions 64:128. P matmuls for head
                    # h+2 are interleaved to cover the exp latency.
                    if h == 0:
                        for it_ in range(NT):
                            emit_v(it_)
                    avacc = ps_av.tile([128, T], f32, name="av_ps", tag="acc", bufs=1)
                    if h + 3 < H:
                        emit_p_pair(h + 3, 0)
                    emit_av_pair(h, attnTw, avacc, [0, 1])
                    if h + 3 < H:
                        emit_p_pair(h + 3, 1)
                    emit_av_pair(h, attnTw, avacc, [2, 3])
                    if h + 3 < H:
                        emit_mods(h + 3)
                    st.pop(h - 1, None)

                    lnden = wk.tile([64, T], f32, name="lnden", tag="lnden")
                    nc.scalar.activation(
                        lnden[:], avacc[64:128, :], AF.Ln, bias=0.0, scale=1.0
                    )
                    rdb = wk.tile([64, T], f32, name="rdb", tag="rdb")
                    nc.scalar.activation(
                        rdb[:], lnden[:], AF.Exp, bias=0.0, scale=-1.0
                    )
                    nc.vector.tensor_mul(
                        avn[hd_tile][hd_off:hd_off + 64, :],
                        avacc[0:64, :],
                        rdb[:],
                    )

                    if h == 5:
                        # W_out partials for ot 0,1 over d-tiles 0..2
                        # (avn[0..2] are complete after this head)
                        st["oacc01"] = ps.tile(
                            [128, 2 * T], f32, name="oacc01", tag="p", bufs=1
                        )
                        for dt in range(3):
                            for o2 in range(2):
                                nc.tensor.matmul(
                                    st["oacc01"][:, o2 * T:(o2 + 1) * T],
                                    wout(dt, o2 * 128, (o2 + 1) * 128),
                                    avn[dt][:],
                                    start=(dt == 0),
                                    stop=False,
                                    skip_group_check=True,
                                )
                    if h == 7:
                        for o2 in range(2):
                            st[f"oacc2{o2}"] = ps.tile(
                                [128, T], f32, name=f"oacc2{o2}", tag="e",
                                bufs=5
                            )
                        for dt in range(3):
                            for o2 in range(2):
                                nc.tensor.matmul(
                                    st[f"oacc2{o2}"][:],
                                    wout(dt, (2 + o2) * 128, (3 + o2) * 128),
                                    avn[dt][:],
                                    start=(dt == 0),
                                    stop=False,
                                    skip_group_check=True,
                                )

                # ---- output projection: finish dt=3 and write out -----
                for pair, tname in ((0, "oacc01"), (1, "oacc23")):
                    for o2 in range(2):
                        ot = pair * 2 + o2
                        oacc_sl = (
                            st["oacc01"][:, o2 * T:(o2 + 1) * T]
                            if pair == 0 else st[f"oacc2{o2}"][:]
                        )
                        nc.tensor.matmul(
                            oacc_sl,
                            wout(3, ot * 128, (ot + 1) * 128),
                            avn[3][:],
                            start=False,
                            stop=True,
                            skip_group_check=True,
                        )
                        nc.scalar.activation(
                            outw[:, ot * T:(ot + 1) * T],
                            oacc_sl,
                            AF.Identity,
                            bias=bout[ot], scale=1.0,
                        )
                        nc.sync.dma_start(
                            out_d[:, ot * T:(ot + 1) * T],
                            outw[:, ot * T:(ot + 1) * T],
                        )

    _split_multi_waits(nc)
    return nc


def _nmajor(a):
    """(512, 1536) -> (128, 12*4*128): [p, nt*512 + dt*128 + c]
    = a[dt*128 + p, nt*128 + c]."""
    out = np.empty((128, 12, 4, 128), a.dtype)
    for nt in range(12):
        for dt in range(4):
            out[:, nt, dt, :] = a[dt * 128:(dt + 1) * 128,
                                  nt * 128:(nt + 1) * 128]
    return np.ascontiguousarray(out.reshape(128, 6144))


def _coalesce(a):
    """(128*ND, W) -> (128, ND*W): [p, dt*W + c] = a[dt*128 + p, c]."""
    n, w = a.shape
    nd = n // 128
    return np.ascontiguousarray(
        a.reshape(nd, 128, w).transpose(1, 0, 2).reshape(128, nd * w)
    )


def _host_constants():
    freqs = np.exp(
        -np.log(10000.0) * np.arange(HALF, dtype=np.float32) / HALF
    )
    idx = np.arange(T, dtype=np.float32)
    ang = np.outer(freqs, idx)  # (HALF, T)
    sing = np.sin(ang).astype(np.float32)
    cosg = np.cos(ang).astype(np.float32)
    sinw = np.concatenate([sing[0:128], sing[128:256]], axis=1).astype(BF16)
    cosw = np.concatenate([cosg[0:128], cosg[128:256]], axis=1).astype(BF16)
    gt = _coalesce(np.concatenate([cosg, sing], axis=0)).astype(BF16)
    msk = np.concatenate(
        [NEG_BIG * np.eye(128, dtype=np.float32), np.eye(128, dtype=np.float32)],
        axis=1,
    ).astype(BF16)
    return sinw, cosw, gt, msk


def kernel(x, W_qkv, W_pos, pos_u, pos_v, W_out, b_out, log_one_div_by_tau):
    from concourse import bass_utils

    if "nc" not in _CACHE:
        _CACHE["nc"] = _build()
        _CACHE["consts"] = _host_constants()
    nc = _CACHE["nc"]
    sinw, cosw, gt, msk = _CACHE["consts"]

    x = np.asarray(x, np.float32)
    sml = np.zeros((128, 13), np.float32)
    for i in range(ND):
        sml[:, i] = np.asarray(pos_u, np.float32).reshape(D)[
            i * 128:(i + 1) * 128
        ]
        sml[:, 4 + i] = np.asarray(pos_v, np.float32).reshape(D)[
            i * 128:(i + 1) * 128
        ]
        sml[:, 8 + i] = np.asarray(b_out, np.float32).reshape(D)[
            i * 128:(i + 1) * 128
        ]
    sml[:, 12] = np.float32(np.asarray(log_one_div_by_tau).reshape(-1)[0])

    shared = {
        "wqkv": _nmajor(np.asarray(W_qkv, np.float32)).astype(BF16),
        "wpT": _coalesce(
            np.ascontiguousarray(np.asarray(W_pos, np.float32).T)
        ).astype(BF16),
        "wout": _coalesce(np.asarray(W_out, np.float32)).astype(BF16),
        "sml": sml,
        "sinw": sinw, "cosw": cosw, "gt": gt, "msk": msk,
    }
    in_maps = []
    for b in range(B):
        m = dict(shared)
        m["xT"] = _coalesce(
            np.ascontiguousarray(x[:, b, :].T)
        ).astype(BF16)
        in_maps.append(m)

    _CACHE["last_in_maps"] = in_maps
    res = bass_utils.run_bass_kernel_spmd(nc, in_maps, core_ids=list(range(B)))
    out = np.empty((T, B, D), np.float32)
    for b in range(B):
        o = res.results[b]["out"]  # (128, ND*T)
        out[:, b, :] = (
            o.reshape(128, ND, T).transpose(1, 0, 2).reshape(D, T).T
        )
    return out
